# revision 40
# baseline (speedup 1.0000x reference)
"""Bahdanau-attention kernel for TRN2, data-parallel over batch on 8 NeuronCores.

Reference computation (B=64, S=1024, H=512):
    energy    = tanh(cat([hidden bcast S, enc], -1) @ attn_w.T + attn_b)  [B,S,H]
    attention = energy @ v_w.T                                            [B,S]
    out       = softmax(where(mask==0, -1e10, attention), axis=1)

Strategy vs the fp32r baseline (107 us):
  1. Masked positions (about half) produce exactly 0 in the output, so the
     host gathers only the unmasked s per batch row, padded to a per-row
     quota SQ=576 (4 sigma above the Binomial(1024,.5) mean; rows that
     overflow trigger a lazily-compiled larger build: 640, then 1024).
     36 row-tiles instead of 64.
  2. The energy matmul runs in fp8 e4m3 with MatmulPerfMode.DoubleRow
     (~0.5 cycles/row).  e4m3 quantization is scale-free, so W/c are cast
     directly.  The hidden-term c (host fp32) is folded into each tile's
     PSUM as a third DoubleRow matmul whose two k-planes carry fp8(c) and
     the fp8 residual, so c lands at ~e4m3^2 precision.
  3. tanh on ACT; the v-dot (scalar_tensor_tensor with accum) on DVE.
     A dozen warm-up matmuls run during the DMA prologue so the PE clock
     is ramped when the real tiles start.
  4. Softmax epilogue without max subtraction (|att| <= |v|_1 < 23).
"""
import numpy as np
import ml_dtypes

import concourse.bass as bass
import concourse.tile as tile
from concourse import bacc, mybir
from concourse.bass_utils import run_bass_kernel_spmd

B, S, H = 64, 1024, 512
NCORES = 8
BLOC = B // NCORES              # 8 batch rows per core
SQ_FAST = 576                   # per-batch-row gathered quota
SLO = 16                        # s-positions per tile per batch row
TPB = 4                         # tiles per DMA block
STT_MODE = "f32"                # "f32" | "f16" | "ttr"
NWARM = 13                      # warm-up matmuls during the DMA prologue
PAIR_ACT = True                 # one tanh per two psum banks: amortizes the
                                # ~357ns fixed ACT cost; DVE paces the loop
HOST_DIV = False                # ship exponentials + denominators, divide
                                # on host (drops reduce/recip/scale from the
                                # device tail); gathered builds only
F32 = mybir.dt.float32
F16 = mybir.dt.float16
BF16 = mybir.dt.bfloat16
FP8 = mybir.dt.float8e4
AF = mybir.ActivationFunctionType
ALU = mybir.AluOpType
DR = mybir.MatmulPerfMode.DoubleRow

_CACHE = {}


def _build(sq, stt_mode=None, nwarm=None, nwax=6, pair_act=None,
           epi_olap=False, host_div=HOST_DIV):
    stt_mode = STT_MODE if stt_mode is None else stt_mode
    nwarm = NWARM if nwarm is None else nwarm
    pair_act = PAIR_ACT if pair_act is None else pair_act
    nt = sq // SLO              # att columns (= row-tiles)
    r = sq * BLOC               # rows per core
    ntile = r // 128
    nblk = ntile // TPB
    assert ntile == nt and ntile % TPB == 0

    tdt = {"f32": F32, "f16": F16, "ttr": F32}[stt_mode]

    nc = bacc.Bacc(None)
    enc8 = nc.dram_tensor("enc8", [128, 2, 2, r], FP8, kind="ExternalInput")
    wf8 = nc.dram_tensor("wf8", [128, 2, 2, H], FP8, kind="ExternalInput")
    c2 = nc.dram_tensor("c2", [128, 2, H], FP8, kind="ExternalInput")
    sel2 = nc.dram_tensor("sel2", [128, 2, 128], FP8, kind="ExternalInput")
    v2 = nc.dram_tensor("v2", [1, H], F32, kind="ExternalInput")
    v2h = nc.dram_tensor("v2h", [1, H], F16, kind="ExternalInput")
    mask01 = nc.dram_tensor("mask01", [128, nt], F32, kind="ExternalInput")
    ident = nc.dram_tensor("ident", [128, 128], BF16, kind="ExternalInput")
    sel8 = nc.dram_tensor("sel8", [128, BLOC], BF16, kind="ExternalInput")
    out = nc.dram_tensor("out", [BLOC, sq], F32, kind="ExternalOutput")

    with tile.TileContext(nc) as tc:
        with tc.tile_pool(name="singles", bufs=1) as singles, \
             tc.tile_pool(name="enc", bufs=nblk) as encp, \
             tc.tile_pool(name="work", bufs=4) as work, \
             tc.tile_pool(name="ps", bufs=2 if pair_act else 5,
                          space="PSUM") as ps, \
             tc.tile_pool(name="pse", bufs=1, space="PSUM") as pse:

            # --- warm-up: ramp engine clocks while the input DMAs land.
            # Counterintuitively a LONG warm-up wins even though it delays
            # tile 0 by a few us: the ramped clocks speed up the whole loop.
            warm_w = singles.tile([128, 2, 16], FP8, tag="warmw")
            warm_r = singles.tile([128, 2, H], FP8, tag="warmr")
            nc.vector.memset(warm_w, 0.0)
            nc.vector.memset(warm_r, 0.0)
            warm_ps = pse.tile([128, H], F32, tag="pd", name="warm_ps")
            warm_t = singles.tile([128, H], F32, tag="warmt")
            warm_a = singles.tile([128, 1], F32, tag="warma")
            for i in range(nwarm):
                nc.tensor.matmul(warm_ps[0:16, :], warm_w, warm_r,
                                 start=True, stop=True, perf_mode=DR)
            if nwax:                # also exercise ACT and DVE (idle anyway)
                warm_a2 = singles.tile([128, 1], F32, tag="warma2")
                for i in range(nwax):
                    nc.scalar.activation(warm_t, warm_ps, AF.Tanh)
                    nc.vector.scalar_tensor_tensor(
                        out=warm_a.broadcast_to([128, H]),
                        in0=warm_t, scalar=0.0, in1=warm_t,
                        op0=ALU.bypass, op1=ALU.mult, accum_out=warm_a2)

            # --- input DMAs: tile-0 critical path first -----------------
            enc0_sb = singles.tile([128, 2, 2, 128], FP8, tag="enc0")
            nc.sync.dma_start(out=enc0_sb, in_=enc8.ap()[:, :, :, 0:128])
            wf8_sb = singles.tile([128, 2, 2, H], FP8, tag="wf8")
            nc.sync.dma_start(out=wf8_sb[:, 0], in_=wf8.ap()[:, 0])
            nc.sync.dma_start(out=wf8_sb[:, 1], in_=wf8.ap()[:, 1])
            c2_sb = singles.tile([128, 2, H], FP8, tag="c2")
            nc.gpsimd.dma_start(out=c2_sb, in_=c2[:])
            sel2_sb = singles.tile([128, 2, 128], FP8, tag="sel2")
            nc.gpsimd.dma_start(out=sel2_sb, in_=sel2[:])
            v_sb = singles.tile([128, H], tdt, tag="v")
            v_src = v2h if tdt == F16 else v2
            nc.gpsimd.dma_start(out=v_sb,
                                in_=v_src.ap().partition_broadcast(128))

            enc_view = enc8.ap().rearrange(
                "p g i (blk q) -> p g i blk q", q=TPB * 128)
            enc_sbs = []
            for blk in range(nblk):
                enc_sb = encp.tile([128, 2, 2, TPB * 128], FP8, tag="enc",
                                   name=f"enc_b{blk}")
                enc_sbs.append(enc_sb)
                nc.sync.dma_start(out=enc_sb, in_=enc_view[:, :, :, blk, :])

            mask_sb = singles.tile([128, nt], F32, tag="mask")
            nc.gpsimd.dma_start(out=mask_sb, in_=mask01[:])
            ident_sb = singles.tile([128, 128], BF16, tag="ident")
            nc.gpsimd.dma_start(out=ident_sb, in_=ident[:])
            sel8_sb = singles.tile([128, BLOC], BF16, tag="sel8")
            nc.gpsimd.dma_start(out=sel8_sb, in_=sel8[:])

            att_all = singles.tile([128, nt], F32, tag="att")
            dummy = singles.tile([128, 1], tdt, tag="dummy")
            prod_sb = work.tile([128, H], F32, tag="prod", bufs=2) \
                if stt_mode == "ttr" else None

            def enc_ap(t):
                if t == 0:
                    return enc0_sb
                blk, tl = t // TPB, t % TPB
                return enc_sbs[blk][:, :, :, tl * 128:(tl + 1) * 128]

            def mm_tile(t, psum_slice):
                src = enc_ap(t)
                for g in range(2):
                    nc.tensor.matmul(
                        psum_slice, src[:, g], wf8_sb[:, g],
                        start=(g == 0), stop=False, perf_mode=DR)
                nc.tensor.matmul(psum_slice, sel2_sb, c2_sb,
                                 start=False, stop=True, perf_mode=DR)

            def stt_tile(t, tanh_ap):
                nc.vector.scalar_tensor_tensor(
                    out=dummy.broadcast_to([128, H]),
                    in0=tanh_ap, scalar=0.0, in1=v_sb,
                    op0=ALU.bypass, op1=ALU.mult,
                    accum_out=att_all[:, t:t + 1],
                )

            # --- main loop ----------------------------------------------
            if pair_act:
                for tp in range(ntile // 2):
                    psum2 = ps.tile([128, 2, H], F32, tag="pe", name="pe2")
                    mm_tile(2 * tp, psum2[:, 0])
                    mm_tile(2 * tp + 1, psum2[:, 1])
                    tanh2 = work.tile([128, 2, H], tdt, tag="tanh",
                                      name="tanh2")
                    nc.scalar.activation(tanh2, psum2, AF.Tanh)
                    stt_tile(2 * tp, tanh2[:, 0])
                    stt_tile(2 * tp + 1, tanh2[:, 1])
            else:
                for t in range(ntile):
                    psum = ps.tile([128, H], F32, tag="pe", name="pe1")
                    mm_tile(t, psum)
                    tanh_sb = work.tile([128, H], tdt, tag="tanh",
                                        name="tanh1")
                    nc.scalar.activation(tanh_sb, psum, AF.Tanh)
                    stt_tile(t, tanh_sb)

            # --- softmax epilogue (no max subtraction) ------------------
            # With gathering, the only masked rows are the padding the host
            # scatter drops, so the output reconstruction can read the raw
            # exponentials and run concurrently with the mask/denominator
            # chain.  The ungathered fallback reconstructs from the masked
            # exponentials (masked positions must output exactly 0).
            gathered = (sq != S) and (epi_olap or host_div)
            e_all = singles.tile([128, nt], BF16 if gathered else F32,
                                 tag="e_all")
            nc.scalar.activation(e_all, att_all, AF.Exp)
            em = singles.tile([128, nt], BF16, tag="em")
            nc.vector.tensor_tensor(out=em, in0=e_all, in1=mask_sb,
                                    op=ALU.mult)
            rsrc = e_all if gathered else em

            # one [8, 16*64] psum: slice stride padded to 64 cols so no
            # matmul output crosses the 2KB bank boundary (sl 8 lands at it)
            psum_o = pse.tile([BLOC, SLO * 64], F32, tag="po", name="po")
            for sl in range(SLO):
                nc.tensor.matmul(
                    psum_o[:, sl * 64:sl * 64 + nt],
                    ident_sb[:, sl * BLOC:(sl + 1) * BLOC],
                    rsrc, start=True, stop=True)

            psum_d = pse.tile([BLOC, nt], F32, tag="pd")
            nc.tensor.matmul(psum_d, sel8_sb, em, start=True, stop=True)

            den8 = singles.tile([BLOC, 1], F32, tag="den8")
            nc.vector.tensor_reduce(den8, psum_d, mybir.AxisListType.X,
                                    ALU.add)
            r8 = singles.tile([BLOC, 1], F32, tag="r8")
            nc.vector.reciprocal(r8, den8)

            out_sb = singles.tile([BLOC, sq], F32, tag="out")
            outv = out_sb.rearrange("p (t sl) -> p t sl", sl=SLO)
            nc.vector.tensor_scalar(
                out=outv,
                in0=psum_o.rearrange("p (sl t) -> p t sl",
                                     sl=SLO)[:, 0:nt, :],
                scalar1=r8, scalar2=None, op0=ALU.mult,
            )
            hq = sq // 2
            nc.sync.dma_start(out=out.ap()[:, 0:hq], in_=out_sb[:, 0:hq])
            nc.gpsimd.dma_start(out=out.ap()[:, hq:sq],
                                in_=out_sb[:, hq:sq])
    nc.finalize()
    return nc


def _get_nc(sq):
    if sq not in _CACHE:
        _CACHE[sq] = _build(sq)
    return _CACHE[sq]


def _fp8(x):
    return np.clip(x, -240.0, 240.0).astype(ml_dtypes.float8_e4m3fn)


def _prep(hidden, encoder_outputs, attn_mask, attn_w, attn_b, v_w):
    """Host-side gather/shard prep.  Returns (sq, in_maps, sidx_list, counts)."""
    hidden = np.asarray(hidden, np.float32)
    enc = np.asarray(encoder_outputs, np.float32)        # [S, B, H]
    mask = np.asarray(attn_mask)
    attn_w = np.asarray(attn_w, np.float32)              # [H, 2H]
    attn_b = np.asarray(attn_b, np.float32)
    v_w = np.asarray(v_w, np.float32).reshape(1, H)

    sidx_list = [np.nonzero(mask[b] != 0)[0] for b in range(B)]
    counts = np.array([len(s) for s in sidx_list])
    cmax = counts.max()
    sq = next(q for q in (SQ_FAST, 640, S) if cmax <= q)
    nt = sq // SLO

    wet = attn_w[:, H:].T                                # [k, h]
    wf8 = _fp8(wet.reshape(2, 2, 128, H).transpose(2, 0, 1, 3))
    c_all = hidden @ attn_w[:, :H].T + attn_b            # [B, H] fp32
    sel2 = np.zeros((128, 2, 128), ml_dtypes.float8_e4m3fn)
    for m in range(128):
        sel2[m % BLOC, :, m] = 1.0
    v2 = v_w.astype(np.float32)
    ident = np.eye(128).astype(ml_dtypes.bfloat16)
    sel8 = np.tile(np.eye(BLOC), (SLO, 1)).astype(ml_dtypes.bfloat16)

    in_maps = []
    for core in range(NCORES):
        enc_g = np.zeros((BLOC, sq, H), np.float32)
        valid = np.zeros((BLOC, sq), np.float32)
        for bl in range(BLOC):
            b = core * BLOC + bl
            if sq == S:
                enc_g[bl] = enc[:, b, :]
                valid[bl] = (mask[b] != 0)
            else:
                idx = sidx_list[b]
                enc_g[bl, :len(idx)] = enc[idx, b, :]
                valid[bl, :len(idx)] = 1.0
        enc8 = _fp8(enc_g.reshape(BLOC, sq, 2, 2, 128)
                    .transpose(4, 2, 3, 1, 0)
                    .reshape(128, 2, 2, BLOC * sq))
        m01 = np.ascontiguousarray(
            valid.reshape(BLOC, nt, SLO)
            .transpose(2, 0, 1)
            .reshape(128, nt))
        cc = c_all[core * BLOC:(core + 1) * BLOC]        # [8, H]
        c_hi = _fp8(cc)
        c_lo = _fp8(cc - c_hi.astype(np.float32))
        c2 = np.zeros((128, 2, H), ml_dtypes.float8_e4m3fn)
        c2[:BLOC, 0] = c_hi
        c2[:BLOC, 1] = c_lo
        in_maps.append({
            "enc8": enc8, "wf8": wf8, "c2": c2, "sel2": sel2,
            "v2": v2, "v2h": v2.astype(ml_dtypes.float16
                                       if hasattr(ml_dtypes, "float16")
                                       else np.float16),
            "mask01": m01, "ident": ident, "sel8": sel8,
        })
    return sq, in_maps, sidx_list, counts


def _assemble(res, sq, sidx_list, counts):
    host_div = HOST_DIV and sq != S
    nt = sq // SLO
    out = np.zeros((B, S), np.float32)
    for core in range(NCORES):
        if host_div:
            po = np.asarray(res.results[core]["po8"], np.float32)
            den = np.asarray(res.results[core]["pd8"],
                             np.float32).sum(axis=1)     # [8]
            den = np.maximum(den, 1e-30)
            og = (po.reshape(BLOC, SLO, 64)[:, :, :nt]
                  .transpose(0, 2, 1).reshape(BLOC, sq)
                  / den[:, None])
        else:
            og = res.results[core]["out"]                # [8, sq]
        for bl in range(BLOC):
            b = core * BLOC + bl
            n = counts[b]
            if n == 0:
                # reference: all logits -1e10 -> softmax is uniform
                out[b, :] = 1.0 / S
            elif sq == S:
                out[b, :] = og[bl]           # masked positions are 0 already
            else:
                out[b, sidx_list[b]] = og[bl, :n]
    return out


def kernel(t, hidden, encoder_outputs, attn_mask, src_gps_seqs, src,
           src_rids, input_id, trg_gps_seqs, attn_w, attn_b, v_w):
    sq, in_maps, sidx_list, counts = _prep(
        hidden, encoder_outputs, attn_mask, attn_w, attn_b, v_w)
    nc = _get_nc(sq)
    res = run_bass_kernel_spmd(nc, in_maps, core_ids=list(range(NCORES)))
    return _assemble(res, sq, sidx_list, counts)
